# revision 11
# baseline (speedup 1.0000x reference)
"""Trainium2 Bass kernel for nn_NeuralNetwork_65618510348896 (binarized MLP).

Network (training-mode BatchNorm, B=65536):
  h1 = x @ sign(W1).T + b1 ; h1c = clip(bn1(h1), -1, 1)
  h2 = sign(h1c) @ sign(W2).T + b2 ; h2c = clip(bn2(h2), -1, 1)
  h3 = sign(h2c) @ sign(W3).T + b3 ; h3c = clip(bn3(h3), -1, 1)
  out = log_softmax(h3c @ W4.T + b4)

Strategy: pure data parallel over 8 NeuronCores (8192 rows each), BatchNorm
statistics via AllReduce of per-core (sum h, sum h^2).

Key transforms vs a direct implementation:
- x is transposed feature-major on the host and pre-rounded to the fp22 grid;
  fc1 runs as a single float32r matmul pass (full PE rate at N=512).
- sign(bn(h)) only needs a per-feature threshold: sign(sc*(h-mean)+be) with
  sc>0 is (h >= t), t = mean - be*std/g.  The fc bias b cancels inside
  training-mode BN.  The comparison output u in {0,1} feeds the next layer
  with weights 2*sign(W); the resulting constant shift per output feature is
  absorbed by that layer's own batch-mean.  All of fc2/fc3 is exact integer
  arithmetic in fp8 DoubleRow matmuls (2 MACs/cell/cycle).
- clip(bn3(h3)) is computed as clip(h3', lo_k, hi_k) with per-feature bounds;
  the per-feature scale sc3 is folded into W4 and the shift into b4 (one tiny
  matmul), so phase 4 needs no scalar-engine work besides exp/ln.
- h2/h3 stay resident in SBUF as fp8 (exact for the threshold/clip uses);
  stats (sum h, sum h^2) accumulate from PSUM in fp32.
- sqrt-free BN: std = exp(+-0.5*ln(var+eps)) keeps every activation in one
  hardware table set (no 1.3us table reloads).
"""
import sys
sys.path.insert(0, "/opt/trn_rl_repo")
sys.path.insert(0, "/root/.axon_site/_ro/trn_rl_repo")

import numpy as np

N_CORES = 8
B_TOT = 65536
BC = B_TOT // N_CORES          # rows per core
NB = 512                       # batch block (moving free dim)
FIN1 = 896                     # 784 padded to 7*128
F1 = 512                       # 500 padded
F2 = 1024
F3 = 1024
F4 = 16                        # 10 padded
BN_EPS = 1e-5

_CACHE = {}


def build(nblk, phases=4):
    import concourse.bass as bass
    import concourse.mybir as mybir
    import concourse.tile as tile
    from concourse import bacc
    from concourse.masks import make_identity

    f32 = mybir.dt.float32
    f32r = mybir.dt.float32r
    f16 = mybir.dt.float16
    bf16 = mybir.dt.bfloat16
    f8 = mybir.dt.float8e4
    AF = mybir.ActivationFunctionType
    OP = mybir.AluOpType
    DR = mybir.MatmulPerfMode.DoubleRow
    bc = nblk * NB

    nc = bacc.Bacc("TRN2", target_bir_lowering=False, debug=False,
                   num_devices=N_CORES)

    xt_t = nc.dram_tensor("xt", [FIN1, bc], f32r, kind="ExternalInput")
    w1_t = nc.dram_tensor("w1t", [FIN1, F1], f32r, kind="ExternalInput")
    w2_t = nc.dram_tensor("w2p", [2, 128, 2, F2], f8, kind="ExternalInput")
    w3_t = nc.dram_tensor("w3p", [4, 128, 2, F3], f8, kind="ExternalInput")
    w4_t = nc.dram_tensor("w4t", [F3, F4], f16, kind="ExternalInput")
    vec_names = ["g1", "be1", "g2", "be2", "g3", "be3"]
    vec_sizes = [F1, F1, F2, F2, F3, F3]
    vec_t = {n: nc.dram_tensor(n, [s], f32, kind="ExternalInput")
             for n, s in zip(vec_names, vec_sizes)}
    b4_t = nc.dram_tensor("b4", [F4], f32, kind="ExternalInput")
    c2_t = nc.dram_tensor("c2", [F2], f32, kind="ExternalInput")
    c3_t = nc.dram_tensor("c3", [F3], f32, kind="ExternalInput")
    out_t = nc.dram_tensor("out", [bc, 16], f32, kind="ExternalOutput")

    K1, M1, M2, M3 = FIN1 // 128, F1 // 128, F2 // 128, F3 // 128
    P2, P3 = M1 // 2, M2 // 2   # DoubleRow k-pair counts for fc2 / fc3

    with tile.TileContext(nc) as tc:
        import contextlib
        ctx = contextlib.ExitStack()
        with ctx:
            # SBUF pools (bytes per partition):
            # actA: h1 fp16 [128,512] x64 then h3' fp8 [128,2,512] x64 -> 64K
            # actB: h2' fp8 pair tiles [128,2,512] x64               -> 64K
            actA = ctx.enter_context(tc.tile_pool(name="actA", bufs=64))
            actB = ctx.enter_context(tc.tile_pool(name="actB", bufs=64))
            xtp = ctx.enter_context(tc.tile_pool(name="xtp", bufs=8))
            utp = ctx.enter_context(tc.tile_pool(name="utp", bufs=6))
            chp = ctx.enter_context(tc.tile_pool(name="chp", bufs=10))
            sqp = ctx.enter_context(tc.tile_pool(name="sqp", bufs=3))
            wp = ctx.enter_context(tc.tile_pool(name="wp", bufs=1))
            cst = ctx.enter_context(tc.tile_pool(name="cst", bufs=1))
            hfp = ctx.enter_context(tc.tile_pool(name="hfp", bufs=2))
            sml = ctx.enter_context(tc.tile_pool(name="sml", bufs=8))
            psA = ctx.enter_context(tc.tile_pool(name="psA", bufs=5,
                                                 space="PSUM"))
            ps4 = ctx.enter_context(tc.tile_pool(name="ps4", bufs=1,
                                                 space="PSUM"))
            pst = ctx.enter_context(tc.tile_pool(name="pstp", bufs=1,
                                                 space="PSUM"))
            drp = ctx.enter_context(tc.tile_pool(name="drp", bufs=1,
                                                 space="DRAM"))

            # ---------------- weight / vector prep ----------------
            w1T = []
            for k in range(K1):
                t = wp.tile([128, F1], f32r, name=f"w1T_{k}", tag=f"w1T_{k}")
                nc.sync.dma_start(t, w1_t[k * 128:(k + 1) * 128, :])
                w1T.append(t)
            w2T = []
            for t_ in range(P2):
                t = wp.tile([128, 2, F2], f8, name=f"w2T_{t_}", tag=f"w2T_{t_}")
                nc.sync.dma_start(t, w2_t[t_])
                w2T.append(t)
            w3T = []
            for t_ in range(P3):
                t = wp.tile([128, 2, F3], f8, name=f"w3T_{t_}", tag=f"w3T_{t_}")
                nc.sync.dma_start(t, w3_t[t_])
                w3T.append(t)
            w4T = []
            for k in range(M3):
                t = wp.tile([128, F4], f16, name=f"w4T_{k}", tag=f"w4T_{k}")
                nc.sync.dma_start(t, w4_t[k * 128:(k + 1) * 128, :])
                w4T.append(t)

            def load_vec(name, nchunk):
                v = cst.tile([128, nchunk], f32, name=f"v_{name}",
                             tag=f"v_{name}")
                nc.sync.dma_start(
                    v, vec_t[name].rearrange("(c p) -> p c", p=128))
                return v

            vg1, vbe1 = load_vec("g1", M1), load_vec("be1", M1)
            vg2, vbe2 = load_vec("g2", M2), load_vec("be2", M2)
            vg3, vbe3 = load_vec("g3", M3), load_vec("be3", M3)
            def load_cvec(tsr, nchunk, nm):
                v = cst.tile([128, nchunk], f32, name=f"cv{nm}", tag=f"cv{nm}")
                nc.sync.dma_start(v, tsr.rearrange("(c p) -> p c", p=128))
                n = cst.tile([128, nchunk], f32, name=f"ncv{nm}",
                             tag=f"ncv{nm}")
                nc.vector.tensor_scalar_mul(n, v, -1.0)
                return v, n

            c2v, nc2v = load_cvec(c2_t, M2, 2)
            c3v, nc3v = load_cvec(c3_t, M3, 3)
            vb4 = cst.tile([16, 1], f32)
            nc.sync.dma_start(vb4, b4_t.rearrange("(p o) -> p o", o=1))
            ident = cst.tile([16, 16], f32)
            make_identity(nc, ident)

            # 1/max(g, tiny): finite even for zero-padded gamma
            def ginv_of(gv, nch, nm):
                gi = cst.tile([128, nch], f32, name=f"gi{nm}", tag=f"gi{nm}")
                nc.vector.tensor_scalar(out=gi, in0=gv, scalar1=1e-30,
                                        scalar2=None, op0=OP.max)
                nc.vector.reciprocal(gi, gi)
                return gi

            gi1, gi2, gi3 = ginv_of(vg1, M1, 1), ginv_of(vg2, M2, 2), \
                ginv_of(vg3, M3, 3)

            # stats accumulators: one column per (feature-tile, block)
            s1c = cst.tile([128, M1 * nblk], f32)
            s2c = cst.tile([128, M2 * nblk], f32)
            s3c = cst.tile([128, M3 * nblk], f32)
            s1q = cst.tile([128, M1 * nblk], f32)
            s2q = cst.tile([128, M2 * nblk], f32)
            s3q = cst.tile([128, M3 * nblk], f32)
            st1 = cst.tile([128, 2 * M1], f32)
            st2 = cst.tile([128, 2 * M2], f32)
            st3 = cst.tile([128, 2 * M3], f32)

            # ---------------- phase 1: fc1 (f32r) + stats ----------------
            h1tiles = [[None] * M1 for _ in range(nblk)]
            for b in range(nblk if phases >= 1 else 0):
                xts = []
                for k in range(K1):
                    xv = xtp.tile([128, NB], f32r, name="xv", tag="xt")
                    nc.sync.dma_start(
                        xv, xt_t[k * 128:(k + 1) * 128, b * NB:(b + 1) * NB])
                    xts.append(xv)
                pss = [psA.tile([128, NB], f32, name="ps1", tag="mm")
                       for _ in range(M1)]
                for k in range(K1):
                    for m in range(M1):
                        nc.tensor.matmul(pss[m],
                                         w1T[k][:, m * 128:(m + 1) * 128],
                                         xts[k], start=(k == 0),
                                         stop=(k == K1 - 1))
                for m in range(M1):
                    c = m * nblk + b
                    h1 = actA.tile([128, NB], f16, name="h1", tag="act")
                    nc.vector.tensor_scalar(
                        out=h1, in0=pss[m], scalar1=0.0, scalar2=None,
                        op0=OP.add, op1=OP.add, accum_out=s1c[:, c:c + 1])
                    sq = sqp.tile([128, NB], bf16, name="sq", tag="sq")
                    nc.scalar.activation(sq, h1, AF.Square,
                                         accum_out=s1q[:, c:c + 1])
                    h1tiles[b][m] = h1

            # ---------------- boundary: allreduce + thresholds ----------
            def boundary(scol, sqcol, st, nch, gv, bev, gi, nm, want_clip):
                for m in range(nch):
                    nc.vector.tensor_reduce(
                        out=st[:, m:m + 1],
                        in_=scol[:, m * nblk:(m + 1) * nblk],
                        axis=mybir.AxisListType.X, op=OP.add)
                    nc.vector.tensor_reduce(
                        out=st[:, nch + m:nch + m + 1],
                        in_=sqcol[:, m * nblk:(m + 1) * nblk],
                        axis=mybir.AxisListType.X, op=OP.add)
                ari = drp.tile([128, 2 * nch], f32, name=f"ari{nm}",
                               tag=f"ari{nm}")
                aro = drp.tile([128, 2 * nch], f32, name=f"aro{nm}",
                               tag=f"aro{nm}", addr_space="Shared")
                nc.sync.dma_start(ari, st)
                nc.gpsimd.collective_compute(
                    "AllReduce", OP.add,
                    replica_groups=[list(range(N_CORES))],
                    ins=[ari.opt()], outs=[aro.opt()])
                stg = cst.tile([128, 2 * nch], f32, name=f"stg{nm}",
                               tag=f"stg{nm}")
                nc.sync.dma_start(stg, aro)
                inv_b = 1.0 / (N_CORES * nblk * NB)
                mean = cst.tile([128, nch], f32, name=f"mean{nm}",
                                tag=f"mean{nm}")
                nc.vector.tensor_scalar_mul(mean, stg[:, 0:nch], inv_b)
                var = cst.tile([128, nch], f32, name=f"var{nm}",
                               tag=f"var{nm}")
                nc.vector.tensor_scalar_mul(var, stg[:, nch:2 * nch], inv_b)
                msq = cst.tile([128, nch], f32, name=f"msq{nm}",
                               tag=f"msq{nm}")
                nc.vector.tensor_tensor(out=msq, in0=mean, in1=mean,
                                        op=OP.mult)
                nc.vector.tensor_tensor(out=var, in0=var, in1=msq,
                                        op=OP.subtract)
                nc.vector.tensor_scalar_add(var, var, BN_EPS)
                lnv = cst.tile([128, nch], f32, name=f"lnv{nm}",
                               tag=f"lnv{nm}")
                nc.scalar.activation(lnv, var, AF.Ln)
                std = cst.tile([128, nch], f32, name=f"std{nm}",
                               tag=f"std{nm}")
                nc.scalar.activation(std, lnv, AF.Exp, scale=0.5)
                # threshold t = mean - be*std/g   (sign(sc*h+bi) <=> h >= t)
                t = cst.tile([128, nch], f32, name=f"thr{nm}", tag=f"thr{nm}")
                nc.vector.tensor_tensor(out=t, in0=bev, in1=std, op=OP.mult)
                nc.vector.tensor_tensor(out=t, in0=t, in1=gi, op=OP.mult)
                nc.vector.tensor_tensor(out=t, in0=mean, in1=t,
                                        op=OP.subtract)
                if not want_clip:
                    return t, None, None, None, None
                # clip bounds: lo = mean + (-1-be)*std/g, hi = mean+(1-be)*std/g
                rstd = cst.tile([128, nch], f32, name=f"rstd{nm}",
                                tag=f"rstd{nm}")
                nc.scalar.activation(rstd, lnv, AF.Exp, scale=-0.5)
                sc = cst.tile([128, nch], f32, name=f"sc{nm}", tag=f"sc{nm}")
                nc.vector.tensor_tensor(out=sc, in0=gv, in1=rstd, op=OP.mult)

                def bound(off, label):
                    bnd = cst.tile([128, nch], f32, name=label, tag=label)
                    # (off - be) * std * ginv + mean
                    nc.vector.tensor_scalar(out=bnd, in0=bev, scalar1=-1.0,
                                            scalar2=off, op0=OP.mult,
                                            op1=OP.add)
                    nc.vector.tensor_tensor(out=bnd, in0=bnd, in1=std,
                                            op=OP.mult)
                    nc.vector.tensor_tensor(out=bnd, in0=bnd, in1=gi,
                                            op=OP.mult)
                    nc.vector.tensor_tensor(out=bnd, in0=mean, in1=bnd,
                                            op=OP.add)
                    return bnd

                lo = bound(-1.0, f"lo{nm}")
                hi = bound(1.0, f"hi{nm}")
                # beta = be - sc*mean  (for folding into b4)
                beta = cst.tile([128, nch], f32, name=f"beta{nm}",
                                tag=f"beta{nm}")
                nc.vector.tensor_tensor(out=beta, in0=sc, in1=mean,
                                        op=OP.mult)
                nc.vector.tensor_tensor(out=beta, in0=bev, in1=beta,
                                        op=OP.subtract)
                return t, sc, lo, hi, beta

            if phases >= 1:
                t1, _, _, _, _ = boundary(s1c, s1q, st1, M1, vg1, vbe1, gi1,
                                          1, False)

            # ---------------- phase 2: u1 + fc2 (fp8 DR) + stats ---------
            h2tiles = [[None] * (M2 // 2) for _ in range(nblk)]
            for b in range(nblk if phases >= 2 else 0):
                ups = []
                for t_ in range(P2):
                    u = utp.tile([128, 2, NB], f8, name="u1", tag="u")
                    for i in range(2):
                        k = 2 * t_ + i
                        nc.gpsimd.tensor_scalar(
                            out=u[:, i, :], in0=h1tiles[b][k],
                            scalar1=t1[:, k:k + 1], scalar2=None,
                            op0=OP.is_ge)
                    ups.append(u)
                for j in range(M2):
                    ps = psA.tile([128, NB], f32, name="ps2", tag="mm")
                    for t_ in range(P2):
                        nc.tensor.matmul(
                            ps, w2T[t_][:, :, j * 128:(j + 1) * 128],
                            ups[t_], start=(t_ == 0), stop=(t_ == P2 - 1),
                            perf_mode=DR)
                    jj, ji = j // 2, j % 2
                    if ji == 0:
                        h2tiles[b][jj] = actB.tile([128, 2, NB], f8,
                                                   name="h2", tag="act")
                    h2 = h2tiles[b][jj]
                    c = j * nblk + b
                    if ji == 0:
                        nc.vector.tensor_scalar(
                            out=h2[:, ji, :], in0=ps,
                            scalar1=c2v[:, j:j + 1],
                            scalar2=None, op0=OP.subtract, op1=OP.add,
                            accum_out=s2c[:, c:c + 1])
                    else:
                        nc.scalar.activation(h2[:, ji, :], ps, AF.Identity,
                                             bias=nc2v[:, j:j + 1],
                                             accum_out=s2c[:, c:c + 1])
                    # var2 only enters the threshold scaled by be2; fp8
                    # rounding of h2 is more than accurate enough for it.
                    sq = sqp.tile([128, NB], bf16, name="sq", tag="sq")
                    if ji == 0:
                        nc.scalar.activation(sq, h2[:, ji, :], AF.Square,
                                             accum_out=s2q[:, c:c + 1])
                    else:
                        nc.vector.scalar_tensor_tensor(
                            out=sq, in0=h2[:, ji, :], scalar=0.0,
                            in1=h2[:, ji, :], op0=OP.add, op1=OP.mult,
                            accum_out=s2q[:, c:c + 1])

            if phases >= 2:
                t2, _, _, _, _ = boundary(s2c, s2q, st2, M2, vg2, vbe2, gi2,
                                          2, False)

            # ---------------- phase 3: u2 + fc3 (fp8 DR) + stats ---------
            h3tiles = [[None] * (M3 // 2) for _ in range(nblk)]
            for b in range(nblk if phases >= 3 else 0):
                ups = []
                for t_ in range(P3):
                    u = utp.tile([128, 2, NB], f8, name="u2", tag="u")
                    for i in range(2):
                        k = 2 * t_ + i
                        nc.gpsimd.tensor_scalar(
                            out=u[:, i, :], in0=h2tiles[b][t_][:, i, :],
                            scalar1=t2[:, k:k + 1], scalar2=None,
                            op0=OP.is_ge)
                    ups.append(u)
                for j in range(M3):
                    ps = psA.tile([128, NB], f32, name="ps3", tag="mm")
                    for t_ in range(P3):
                        nc.tensor.matmul(
                            ps, w3T[t_][:, :, j * 128:(j + 1) * 128],
                            ups[t_], start=(t_ == 0), stop=(t_ == P3 - 1),
                            perf_mode=DR)
                    jj, ji = j // 2, j % 2
                    if ji == 0:
                        h3tiles[b][jj] = actA.tile([128, 2, NB], f8,
                                                   name="h3", tag="act")
                    h3 = h3tiles[b][jj]
                    c = j * nblk + b
                    if ji == 0:
                        nc.vector.tensor_scalar(
                            out=h3[:, ji, :], in0=ps,
                            scalar1=c3v[:, j:j + 1],
                            scalar2=None, op0=OP.subtract, op1=OP.add,
                            accum_out=s3c[:, c:c + 1])
                    else:
                        nc.scalar.activation(h3[:, ji, :], ps, AF.Identity,
                                             bias=nc3v[:, j:j + 1],
                                             accum_out=s3c[:, c:c + 1])
                    sq = sqp.tile([128, NB], bf16, name="sq", tag="sq")
                    if ji == 0:
                        nc.scalar.activation(sq, h3[:, ji, :], AF.Square,
                                             accum_out=s3q[:, c:c + 1])
                    else:
                        nc.vector.scalar_tensor_tensor(
                            out=sq, in0=h3[:, ji, :], scalar=0.0,
                            in1=h3[:, ji, :], op0=OP.add, op1=OP.mult,
                            accum_out=s3q[:, c:c + 1])

            if phases >= 3:
                _, sc3, lo3, hi3, beta3 = boundary(s3c, s3q, st3, M3, vg3,
                                                   vbe3, gi3, 3, True)
                # fold sc3 into W4 and beta3 into b4:
                #   logits = sum_k clip(h3',lo,hi)_k * (sc3_k*W4_k)
                #          + (b4 + sum_k beta3_k*W4_k)
                w4s = []
                for k in range(M3):
                    ws = cst.tile([128, F4], f16, name=f"w4s{k}",
                                  tag=f"w4s{k}")
                    nc.vector.tensor_scalar(out=ws, in0=w4T[k],
                                            scalar1=sc3[:, k:k + 1],
                                            scalar2=None, op0=OP.mult)
                    w4s.append(ws)
                beta16 = cst.tile([128, M3], f16)
                nc.vector.tensor_scalar(out=beta16, in0=beta3, scalar1=0.0,
                                        scalar2=None, op0=OP.add)
                psb = pst.tile([16, 1], f32, name="psb", tag="pt")
                for k in range(M3):
                    nc.tensor.matmul(psb, w4T[k], beta16[:, k:k + 1],
                                     start=(k == 0), stop=(k == M3 - 1))
                b4f = cst.tile([16, 1], f32)
                nc.vector.tensor_tensor(out=b4f, in0=vb4, in1=psb, op=OP.add)

            # ---------------- phase 4: clip + fc4 + log_softmax ----------
            for b in range(nblk if phases >= 4 else 0):
                ch3 = []
                for k in range(M3):
                    kk, ki = k // 2, k % 2
                    cl = chp.tile([128, NB], f16, name="cl3", tag="ch")
                    eng = nc.vector if ki == 0 else nc.gpsimd
                    eng.tensor_scalar(out=cl,
                                      in0=h3tiles[b][kk][:, ki, :],
                                      scalar1=lo3[:, k:k + 1],
                                      scalar2=hi3[:, k:k + 1],
                                      op0=OP.max, op1=OP.min)
                    ch3.append(cl)
                ps_l = ps4.tile([16, NB], f32, name="psl", tag="psl")
                for k in range(M3):
                    nc.tensor.matmul(ps_l, w4s[k], ch3[k],
                                     start=(k == 0), stop=(k == M3 - 1))
                lg = hfp.tile([16, NB], f32, name="lg", tag="hf")
                nc.scalar.activation(lg, ps_l, AF.Identity, bias=b4f)
                for r in range(NB // 128):
                    pt = pst.tile([128, 16], f32, name="pt", tag="pt")
                    nc.tensor.transpose(pt, lg[:, r * 128:(r + 1) * 128],
                                        ident)
                    e = sml.tile([128, 16], f32, name="e", tag="sm")
                    nc.scalar.activation(e[:, 0:10], pt[:, 0:10], AF.Exp)
                    se = sml.tile([128, 1], f32, name="se", tag="se")
                    nc.vector.tensor_reduce(out=se, in_=e[:, 0:10],
                                            axis=mybir.AxisListType.X,
                                            op=OP.add)
                    ls = sml.tile([128, 1], f32, name="ls", tag="ls")
                    nc.scalar.activation(ls, se, AF.Ln)
                    o = sml.tile([128, 16], f32, name="o", tag="sm")
                    nc.vector.tensor_scalar(out=o, in0=pt, scalar1=ls,
                                            scalar2=None, op0=OP.subtract)
                    nc.sync.dma_start(
                        out_t[b * NB + r * 128:b * NB + (r + 1) * 128, :], o)

    nc.compile()
    return nc


def _pad(a, shape):
    out = np.zeros(shape, a.dtype)
    out[tuple(slice(0, s) for s in a.shape)] = a
    return out


def _round_fp22(x):
    """Round fp32 to nearest fp22 (e8m13) value so the PE's fp32->fp22
    truncation is exact."""
    u = x.view(np.uint32)
    keep = np.uint32(0xFFFFFC00)
    lsb = (u >> np.uint32(10)) & np.uint32(1)
    u = (u + np.uint32(0x1FF) + lsb) & keep
    return u.view(np.float32)


def prepare_inputs(x, W1, b1, g1, be1, W2, b2, g2, be2, W3, b3, g3, be3,
                   W4, b4, nblk):
    import ml_dtypes
    f8 = ml_dtypes.float8_e4m3
    n = x.shape[0] // N_CORES

    def pack_pairs(sw, np_, fout):
        # sw: [fout, fin] in {-2,0,2}; -> [fin/256, 128, 2, fout]
        swT = np.ascontiguousarray(sw.T)              # [fin, fout]
        return np.ascontiguousarray(
            swT.reshape(np_, 2, 128, fout).transpose(0, 2, 1, 3)).astype(f8)

    sw2 = _pad(2.0 * np.sign(np.asarray(W2, np.float32)), (F2, F1))
    sw3 = 2.0 * np.sign(np.asarray(W3, np.float32))
    common = {
        "w1t": np.ascontiguousarray(
            _pad(np.sign(np.asarray(W1, np.float32)), (F1, FIN1)).T),
        "w2p": pack_pairs(sw2, 2, F2),
        "w3p": pack_pairs(sw3, 4, F3),
        "w4t": np.ascontiguousarray(
            _pad(np.asarray(W4, np.float32), (F4, F3)).T.astype(np.float16)),
        "g1": _pad(np.asarray(g1, np.float32), (F1,)),
        "be1": _pad(np.asarray(be1, np.float32), (F1,)),
        "g2": np.asarray(g2, np.float32), "be2": np.asarray(be2, np.float32),
        "g3": np.asarray(g3, np.float32), "be3": np.asarray(be3, np.float32),
        "b4": _pad(np.asarray(b4, np.float32), (F4,)),
        "c2": np.sign(np.asarray(W2, np.float32)).sum(1),
        "c3": np.sign(np.asarray(W3, np.float32)).sum(1),
    }
    xr = _round_fp22(np.asarray(x, np.float32))
    xT = _pad(xr, (x.shape[0], FIN1)).T               # [FIN1, B]
    return [dict(common,
                 xt=np.ascontiguousarray(xT[:, i * n:(i + 1) * n]))
            for i in range(N_CORES)]


class SpmdRunner:
    """Build-once/run-many executor via PJRT (adapted from
    concourse.bass2jax.run_bass_via_pjrt)."""

    def __init__(self, nc, n_cores):
        import jax
        import concourse.mybir as mybir
        from concourse import bass2jax
        from concourse.bass2jax import _bass_exec_p, install_neuronx_cc_hook
        from jax.sharding import Mesh, PartitionSpec
        from jax.experimental.shard_map import shard_map

        install_neuronx_cc_hook()
        self.jax = jax
        self.nc = nc
        self.n_cores = n_cores
        partition_name = (nc.partition_id_tensor.name
                          if nc.partition_id_tensor else None)
        in_names, out_names, out_avals, zero_outs = [], [], [], []
        for alloc in nc.m.functions[0].allocations:
            if not isinstance(alloc, mybir.MemoryLocationSet):
                continue
            name = alloc.memorylocations[0].name
            if alloc.kind == "ExternalInput":
                if name != partition_name:
                    in_names.append(name)
            elif alloc.kind == "ExternalOutput":
                out_names.append(name)
                shape = tuple(alloc.tensor_shape)
                dtype = mybir.dt.np(alloc.dtype)
                out_avals.append(jax.core.ShapedArray(shape, dtype))
                zero_outs.append(np.zeros(shape, dtype))
        self.in_names = list(in_names)
        self.out_names = out_names
        self.out_avals = out_avals
        self.zero_outs = zero_outs
        n_params = len(in_names)
        n_outs = len(out_avals)
        all_in_names = list(in_names) + list(out_names)
        if partition_name is not None:
            all_in_names.append(partition_name)

        def _body(*args):
            operands = list(args)
            if partition_name is not None:
                operands.append(bass2jax.partition_id_tensor())
            outs = _bass_exec_p.bind(
                *operands,
                out_avals=tuple(out_avals),
                in_names=tuple(all_in_names),
                out_names=tuple(out_names),
                lowering_input_output_aliases=(),
                sim_require_finite=False,
                sim_require_nnan=False,
                nc=nc,
            )
            return tuple(outs)

        devices = jax.devices()[:n_cores]
        mesh = Mesh(np.asarray(devices), ("core",))
        self.mesh = mesh
        self.PartitionSpec = PartitionSpec
        in_specs = (PartitionSpec("core"),) * (n_params + n_outs)
        out_specs = (PartitionSpec("core"),) * n_outs
        self.sharded = jax.jit(
            shard_map(_body, mesh=mesh, in_specs=in_specs,
                      out_specs=out_specs, check_rep=False),
            keep_unused=True,
        )

    def prepare(self, in_maps):
        n = self.n_cores
        args = []
        for name in self.in_names:
            args.append(np.concatenate(
                [np.asarray(in_maps[c][name]) for c in range(n)], axis=0))
        for z in self.zero_outs:
            args.append(np.zeros((n * z.shape[0], *z.shape[1:]), z.dtype))
        from jax.sharding import NamedSharding
        sh = NamedSharding(self.mesh, self.PartitionSpec("core"))
        return [self.jax.device_put(a, sh) for a in args]

    def run(self, dev_args):
        outs = self.sharded(*dev_args)
        self.jax.block_until_ready(outs)
        return outs

    def results(self, outs):
        res = []
        for c in range(self.n_cores):
            d = {}
            for i, name in enumerate(self.out_names):
                d[name] = np.asarray(outs[i]).reshape(
                    self.n_cores, *self.out_avals[i].shape)[c]
            res.append(d)
        return res

    def time_runs(self, dev_args, iters=5, warmup=2):
        import time
        for _ in range(warmup):
            self.run(dev_args)
        ts = []
        for _ in range(iters):
            t0 = time.perf_counter()
            self.run(dev_args)
            ts.append(time.perf_counter() - t0)
        return min(ts)


def get_runner(nblk=BC // NB):
    if nblk not in _CACHE:
        nc = build(nblk)
        _CACHE[nblk] = SpmdRunner(nc, N_CORES)
    return _CACHE[nblk]


def kernel(**inputs) -> np.ndarray:
    r = get_runner()
    in_maps = prepare_inputs(nblk=BC // NB, **inputs)
    dev = r.prepare(in_maps)
    outs = r.run(dev)
    res = r.results(outs)
    return np.concatenate([res[i]["out"][:, 0:10] for i in range(N_CORES)],
                          axis=0)
